# revision 16
# baseline (speedup 1.0000x reference)
"""Additive multi-head attention on 8 Trainium2 NeuronCores.

Sharding: one head per core (H=8); each core processes that head for both
batch elements (b=0 and b=1), which balances work because the two batches
have different key lengths (only k < k_len columns are computed; masked
key columns are exactly zero after softmax and are zero-filled on the
host, and out rows past q_len are zeroed on the host).

Layout: queries are processed in 6 blocks of 64. SBUF partitions hold
(e_sub in {0,1}) x (q_hat in 0..63): p = e_sub*64 + q_hat, with the
remaining 32 "e_major" values of the DC=64 hidden dim on the free axis:

  hid[p, emaj, k] = tanh(kW[k, 2*emaj+e_sub] + qW[64*b+q_hat, 2*emaj+e_sub])

The kW term is replicated across q_hat by the host (kwrep, fp16); the qW
term is either a stride-0 column broadcast (DVE tensor_tensor, short-K
unit) or a per-partition scalar in per-emaj fp16 tensor_scalar ops
(long-K unit, hits the DVE 2x/4x single-src perf modes). The w2
contraction is 32 PSUM-accumulated matmuls per block with a
block-diagonal stationary operand w2d[:, emaj] (128x64, fp16); scores
land as (64, K) full-partition PSUM tiles and softmax reads PSUM
directly. out = att @ v uses PE-transposed att tiles. Units are
processed in interleaved block pairs, ending on the short-K unit to
shorten the dependency tail.
"""

import sys

sys.path.insert(0, "/opt/trn_rl_repo")

import numpy as np

H = 8
DQ = DK = DV = 32
DC = 64
BS = 2
LQ = LK = 384
NBLK = LQ // 64   # 6 query blocks of 64
NEM = DC // 2     # 32 e_major values
ECH = 8           # e_major per kwrep/w2d load chunk

_CACHE = {}


def _ktiles(K):
    tiles = []
    off = 0
    while off < K:
        rows = min(128, K - off)
        tiles.append((off, rows))
        off += rows
    return tiles


def _build_program(K0, K1):
    import concourse.bacc as bacc
    import concourse.mybir as mybir
    import concourse.tile as tile

    f32 = mybir.dt.float32
    f16 = mybir.dt.float16

    nc = bacc.Bacc()
    Ks = (K0, K1)
    Kps = tuple(K + (K % 2) for K in Ks)  # pad odd K for DVE 2x/4x modes
    dins = {}
    douts = {}
    for u in (0, 1):
        K, Kp = Ks[u], Kps[u]
        dins[f"kwrep{u}"] = nc.declare_dram_parameter(
            f"kwrep{u}", [128, NEM * Kp], f16, isOutput=False)
        dins[f"qwb{u}"] = nc.declare_dram_parameter(
            f"qwb{u}", [128, NBLK * NEM], f32, isOutput=False)
        dins[f"v{u}"] = nc.declare_dram_parameter(
            f"v{u}", [K, DV], f32, isOutput=False)
        douts[f"att{u}"] = nc.declare_dram_parameter(
            f"att{u}", [LQ, K], f32, isOutput=True)
        douts[f"o{u}"] = nc.declare_dram_parameter(
            f"o{u}", [LQ, DV], f32, isOutput=True)
    dins["w2d"] = nc.declare_dram_parameter(
        "w2d", [128, NEM * 64], f16, isOutput=False)
    dins["ident"] = nc.declare_dram_parameter(
        "ident", [128, 128], f32, isOutput=False)

    Tanh = mybir.ActivationFunctionType.Tanh
    Exp = mybir.ActivationFunctionType.Exp

    UORDER = (1, 0)  # long-K unit first
    with tile.TileContext(nc) as tc:
        with (
            tc.tile_pool(name="const", bufs=1) as constp,
            tc.tile_pool(name="unit", bufs=1) as unitp,
            tc.tile_pool(name="sums", bufs=3) as sump,
            tc.tile_pool(name="hids", bufs=4) as hidp,
            tc.tile_pool(name="atts", bufs=4) as attp,
            tc.tile_pool(name="small", bufs=6) as smallp,
            tc.tile_pool(name="ps_blk", bufs=3, space="PSUM") as ps_blk,
            tc.tile_pool(name="ps_tr", bufs=2, space="PSUM") as ps_tr,
            tc.tile_pool(name="ps_o", bufs=1, space="PSUM") as ps_o,
        ):
            U = {}
            for u in UORDER:
                K, Kp = Ks[u], Kps[u]
                U[u] = dict(K=K, Kp=Kp, tiles=_ktiles(K))

            # input loads: kwrep of the first unit leads; w2d is chunked
            # so early matmuls aren't gated on the full 512KB
            for u in UORDER:
                Kp = U[u]["Kp"]
                kwrep = unitp.tile([128, NEM * Kp], f16, tag=f"kwrep{u}",
                                   name=f"kwrep{u}")
                for a0 in range(0, NEM, ECH):
                    nc.sync.dma_start(
                        kwrep[:, a0 * Kp:(a0 + ECH) * Kp],
                        dins[f"kwrep{u}"][:, a0 * Kp:(a0 + ECH) * Kp])
                U[u]["kwrep3"] = kwrep[:].rearrange("p (a k) -> p a k", a=NEM)
                qwb = unitp.tile([128, NBLK * NEM], f32, tag=f"qwb{u}",
                                 name=f"qwb{u}")
                nc.sync.dma_start(qwb[:], dins[f"qwb{u}"][:])
                U[u]["qwb3"] = qwb[:].rearrange("p (b a) -> p b a", b=NBLK)

            w2d = constp.tile([128, NEM * 64], f16)
            for a0 in range(0, NEM, ECH):
                nc.sync.dma_start(w2d[:, a0 * 64:(a0 + ECH) * 64],
                                  dins["w2d"][:, a0 * 64:(a0 + ECH) * 64])
            w2d3 = w2d[:].rearrange("p (a m) -> p a m", a=NEM)
            ident = constp.tile([128, 128], f32)
            nc.sync.dma_start(ident[:], dins["ident"][:])

            for u in UORDER:
                v_sb = []
                for t, (off, rows) in enumerate(U[u]["tiles"]):
                    vt = unitp.tile([rows, DV], f32, tag=f"v{u}_{t}",
                                    name=f"v{u}_{t}")
                    nc.sync.dma_start(vt[:], dins[f"v{u}"][off:off + rows, :])
                    v_sb.append(vt)
                U[u]["v_sb"] = v_sb
                attT_sb = []
                for t, (off, rows) in enumerate(U[u]["tiles"]):
                    at = unitp.tile([rows, LQ], f32, tag=f"attT{u}_{t}",
                                    name=f"attT{u}_{t}")
                    attT_sb.append(at)
                U[u]["attT_sb"] = attT_sb

            def make_hid(u, b):
                K, Kp = U[u]["K"], U[u]["Kp"]
                kwrep3, qwb3 = U[u]["kwrep3"], U[u]["qwb3"]
                hid_t = hidp.tile([128, NEM, Kp], f16, tag="hid",
                                  name=f"hid{u}_{b}")
                if K >= 128:
                    # per-emaj fp16 tensor_scalar: single-src DVE perf mode
                    sum_t = sump.tile([128, NEM, Kp], f16, tag="sum16",
                                      name=f"s16_{u}_{b}")
                    for emaj in range(NEM):
                        nc.vector.tensor_scalar_add(
                            sum_t[:, emaj, :],
                            kwrep3[:, emaj, :],
                            qwb3[:, b, emaj:emaj + 1])
                    HC = NEM // 2
                    for a0 in (0, HC):
                        nc.scalar.activation(
                            hid_t[:, a0:a0 + HC, :],
                            sum_t[:, a0:a0 + HC, :], Tanh)
                else:
                    HC = NEM // 2
                    for a0 in (0, HC):
                        sum_t = sump.tile([128, HC, Kp], f32, tag="sum32",
                                          name=f"s32_{u}_{b}_{a0}")
                        nc.vector.tensor_add(
                            sum_t[:],
                            kwrep3[:, a0:a0 + HC, :],
                            U[u]["qwb3"][:, b, :].unsqueeze(2)
                            .broadcast_to([128, NEM, Kp])[:, a0:a0 + HC, :],
                        )
                        nc.scalar.activation(
                            hid_t[:, a0:a0 + HC, :], sum_t[:], Tanh)
                return hid_t

            def do_block_pair(u, bp):
                K, Kp = U[u]["K"], U[u]["Kp"]
                tiles = U[u]["tiles"]
                b0 = 2 * bp
                hid_pair = [make_hid(u, b0), make_hid(u, b0 + 1)]
                ps_pair = ps_blk.tile([128, Kp], f32, tag="blk",
                                      name=f"blk{u}_{bp}",
                                      padded_shape=[128, 512])
                for i in (0, 1):
                    for emaj in range(NEM):
                        nc.tensor.matmul(
                            ps_pair[64 * i:64 * i + 64, :],
                            w2d3[:, emaj, :],
                            hid_pair[i][:, emaj, :],
                            start=(emaj == 0), stop=(emaj == NEM - 1),
                        )
                # softmax along k (valid K columns), straight out of PSUM
                for i in (0, 1):
                    b = b0 + i
                    sc = ps_pair[64 * i:64 * i + 64, 0:K]
                    negmax = smallp.tile([64, 1], f32, tag="negmax")
                    nc.vector.tensor_reduce(
                        negmax[:], sc, axis=mybir.AxisListType.X,
                        op=mybir.AluOpType.max, negate=True)
                    att = attp.tile([64, K], f32, tag="att",
                                    name=f"att{u}_{b}")
                    sums = smallp.tile([64, 1], f32, tag="sums")
                    nc.scalar.activation(att[:], sc, Exp,
                                         bias=negmax[:], accum_out=sums[:])
                    rsum = smallp.tile([64, 1], f32, tag="rsum")
                    nc.vector.reciprocal(rsum[:], sums[:])
                    nc.vector.tensor_scalar_mul(att[:], att[:], rsum[:])
                    nc.gpsimd.dma_start(
                        douts[f"att{u}"][64 * b:64 * b + 64, :], att[:])
                    for t, (off, rows) in enumerate(tiles):
                        trp = ps_tr.tile([rows, 64], f32, tag="tr",
                                         padded_shape=[128, 512])
                        nc.tensor.transpose(
                            trp[:],
                            att[:, off:off + rows],
                            ident[0:64, 0:64],
                        )
                        nc.vector.tensor_copy(
                            U[u]["attT_sb"][t][:, 64 * b:64 * b + 64],
                            trp[:])

            for bp in range(NBLK // 2):
                for u in UORDER:
                    do_block_pair(u, bp)

            for u in UORDER:
                tiles = U[u]["tiles"]
                KT = len(tiles)
                for qt in range(3):
                    op = ps_o.tile([128, DV], f32, tag="o",
                                   padded_shape=[128, 512])
                    for t, (off, rows) in enumerate(tiles):
                        nc.tensor.matmul(
                            op[:],
                            U[u]["attT_sb"][t][:, qt * 128:(qt + 1) * 128],
                            U[u]["v_sb"][t][:],
                            start=(t == 0), stop=(t == KT - 1),
                        )
                    osb = smallp.tile([128, DV], f32, tag="osb")
                    nc.vector.tensor_copy(osb[:], op[:])
                    nc.gpsimd.dma_start(
                        douts[f"o{u}"][qt * 128:(qt + 1) * 128, :], osb[:])

    nc.compile()
    return nc


def _get_program(K0, K1):
    key = (K0, K1)
    if key not in _CACHE:
        _CACHE[key] = _build_program(K0, K1)
    return _CACHE[key]


def _host_prep(q, k, v, w1, b1, w2, k_lens):
    """Per-core input maps. Core i handles head i for b=0 and b=1."""
    ident = np.eye(128, dtype=np.float32)
    in_maps = []
    for h in range(H):
        m = {"ident": ident}
        # w2d[e_sub*64+q_hat, emaj*64+m] = (q_hat==m) * w2[2*emaj+e_sub]
        w2v = w2[h].reshape(NEM, 2)  # [emaj, e_sub]
        z = np.zeros((2, 64, NEM, 64), np.float16)
        idx = np.arange(64)
        for es in range(2):
            z[es, idx, :, idx] = w2v[:, es].astype(np.float16)[None, :]
        m["w2d"] = np.ascontiguousarray(z.reshape(128, NEM * 64))
        for u in range(BS):
            K = int(k_lens[u])
            Kp = K + (K % 2)
            qh = q[u, :, h * DQ:(h + 1) * DQ]          # (384, 32)
            kh = k[u, :K, h * DK:(h + 1) * DK]         # (K, 32)
            qW = qh @ w1[h, :DQ, :] + b1[h]            # (384, 64)
            kW = kh @ w1[h, DQ:, :]                    # (K, 64)
            # kwrep[e_sub*64+q_hat, emaj*Kp+k] = kW[k, 2*emaj+e_sub]
            kw2 = kW.T.reshape(NEM, 2, K).transpose(1, 0, 2)  # (2, NEM, K)
            if Kp != K:
                kw2 = np.concatenate(
                    [kw2, np.zeros((2, NEM, Kp - K), kw2.dtype)], axis=2)
            kwrep = np.broadcast_to(
                kw2[:, None, :, :], (2, 64, NEM, Kp)).reshape(128, NEM * Kp)
            # qwb[e_sub*64+q_hat, b*NEM+emaj] = qW[64b+q_hat, 2emaj+e_sub]
            qwb = (qW.reshape(NBLK, 64, NEM, 2)
                   .transpose(3, 1, 0, 2).reshape(128, NBLK * NEM))
            m[f"kwrep{u}"] = np.ascontiguousarray(kwrep, np.float16)
            m[f"qwb{u}"] = np.ascontiguousarray(qwb, np.float32)
            m[f"v{u}"] = np.ascontiguousarray(
                v[u, :K, h * DV:(h + 1) * DV], np.float32)
        in_maps.append(m)
    return in_maps


def kernel(q, k, v, q_sequence_lengths, k_sequence_lengths, w1, b1, w2):
    from concourse.bass_utils import run_bass_kernel_spmd

    q = np.asarray(q, np.float32)
    k = np.asarray(k, np.float32)
    v = np.asarray(v, np.float32)
    w1 = np.asarray(w1, np.float32)
    b1 = np.asarray(b1, np.float32)
    w2 = np.asarray(w2, np.float32)
    q_lens = np.asarray(q_sequence_lengths).astype(np.int64)
    k_lens = np.asarray(k_sequence_lengths).astype(np.int64)

    K0, K1 = int(k_lens[0]), int(k_lens[1])
    nc = _get_program(K0, K1)
    in_maps = _host_prep(q, k, v, w1, b1, w2, k_lens)
    res = run_bass_kernel_spmd(nc, in_maps, list(range(H))).results

    att = np.zeros((BS, H, LQ, LK), np.float32)
    out = np.zeros((BS, LQ, H * DV), np.float32)
    Ks = (K0, K1)
    for h in range(H):
        for u in range(BS):
            att[u, h, :, :Ks[u]] = res[h][f"att{u}"]
            out[u, :, h * DV:(h + 1) * DV] = res[h][f"o{u}"]
    for u in range(BS):
        out[u, int(q_lens[u]):, :] = 0.0
    return out, att


# revision 17
# speedup vs baseline: 1.0453x; 1.0453x over previous
"""Additive multi-head attention on 8 Trainium2 NeuronCores.

Sharding: one head per core (H=8); each core processes that head for both
batch elements (b=0 and b=1), which balances work because the two batches
have different key lengths (only k < k_len columns are computed; masked
key columns are exactly zero after softmax and are zero-filled on the
host, and out rows past q_len are zeroed on the host).

Layout: queries are processed in 6 blocks of 64. SBUF partitions hold
(e_sub in {0,1}) x (q_hat in 0..63): p = e_sub*64 + q_hat, with the
remaining 32 "e_major" values of the DC=64 hidden dim on the free axis:

  hid[p, emaj, k] = tanh(kW[k, 2*emaj+e_sub] + qW[64*b+q_hat, 2*emaj+e_sub])

The kW term is replicated across q_hat by the host (kwrep, fp16); the qW
term is either a stride-0 column broadcast (DVE tensor_tensor, short-K
unit) or a per-partition scalar in per-emaj fp16 tensor_scalar ops
(long-K unit, hits the DVE 2x/4x single-src perf modes). The w2
contraction is 32 PSUM-accumulated matmuls per block with a
block-diagonal stationary operand w2d[:, emaj] (128x64, fp16); scores
land as (64, K) full-partition PSUM tiles and softmax reads PSUM
directly. out = att @ v uses PE-transposed att tiles. Units are
processed in interleaved block pairs, ending on the short-K unit to
shorten the dependency tail.
"""

import sys

sys.path.insert(0, "/opt/trn_rl_repo")

import numpy as np

H = 8
DQ = DK = DV = 32
DC = 64
BS = 2
LQ = LK = 384
NBLK = LQ // 64   # 6 query blocks of 64
NEM = DC // 2     # 32 e_major values
ECH = 8           # e_major per kwrep/w2d load chunk

_CACHE = {}


def _ktiles(K):
    tiles = []
    off = 0
    while off < K:
        rows = min(128, K - off)
        tiles.append((off, rows))
        off += rows
    return tiles


def _build_program(K0, K1):
    import concourse.bacc as bacc
    import concourse.mybir as mybir
    import concourse.tile as tile

    f32 = mybir.dt.float32
    f16 = mybir.dt.float16

    nc = bacc.Bacc()
    Ks = (K0, K1)
    Kps = tuple(K + (K % 2) for K in Ks)  # pad odd K for DVE 2x/4x modes
    dins = {}
    douts = {}
    for u in (0, 1):
        K, Kp = Ks[u], Kps[u]
        dins[f"kwrep{u}"] = nc.declare_dram_parameter(
            f"kwrep{u}", [128, NEM * Kp], f16, isOutput=False)
        dins[f"qwb{u}"] = nc.declare_dram_parameter(
            f"qwb{u}", [128, NBLK * NEM], f32, isOutput=False)
        dins[f"v{u}"] = nc.declare_dram_parameter(
            f"v{u}", [K, DV], f32, isOutput=False)
        douts[f"att{u}"] = nc.declare_dram_parameter(
            f"att{u}", [LQ, K], f32, isOutput=True)
        douts[f"o{u}"] = nc.declare_dram_parameter(
            f"o{u}", [LQ, DV], f32, isOutput=True)
    dins["w2d"] = nc.declare_dram_parameter(
        "w2d", [128, NEM * 64], f16, isOutput=False)
    dins["ident"] = nc.declare_dram_parameter(
        "ident", [128, 128], f32, isOutput=False)

    Tanh = mybir.ActivationFunctionType.Tanh
    Exp = mybir.ActivationFunctionType.Exp

    UORDER = (1, 0)  # long-K unit first
    with tile.TileContext(nc) as tc:
        with (
            tc.tile_pool(name="const", bufs=1) as constp,
            tc.tile_pool(name="unit", bufs=1) as unitp,
            tc.tile_pool(name="sums", bufs=3) as sump,
            tc.tile_pool(name="hids", bufs=4) as hidp,
            tc.tile_pool(name="atts", bufs=4) as attp,
            tc.tile_pool(name="small", bufs=6) as smallp,
            tc.tile_pool(name="ps_blk", bufs=3, space="PSUM") as ps_blk,
            tc.tile_pool(name="ps_tr", bufs=2, space="PSUM") as ps_tr,
            tc.tile_pool(name="ps_o", bufs=1, space="PSUM") as ps_o,
        ):
            U = {}
            for u in UORDER:
                K, Kp = Ks[u], Kps[u]
                U[u] = dict(K=K, Kp=Kp, tiles=_ktiles(K))

            # input loads: kwrep of the first unit leads; w2d is chunked
            # so early matmuls aren't gated on the full 512KB
            for u in UORDER:
                Kp = U[u]["Kp"]
                kwrep = unitp.tile([128, NEM * Kp], f16, tag=f"kwrep{u}",
                                   name=f"kwrep{u}")
                for a0 in range(0, NEM, ECH):
                    nc.sync.dma_start(
                        kwrep[:, a0 * Kp:(a0 + ECH) * Kp],
                        dins[f"kwrep{u}"][:, a0 * Kp:(a0 + ECH) * Kp])
                U[u]["kwrep3"] = kwrep[:].rearrange("p (a k) -> p a k", a=NEM)
                qwb = unitp.tile([128, NBLK * NEM], f32, tag=f"qwb{u}",
                                 name=f"qwb{u}")
                nc.sync.dma_start(qwb[:], dins[f"qwb{u}"][:])
                U[u]["qwb3"] = qwb[:].rearrange("p (b a) -> p b a", b=NBLK)

            w2d = constp.tile([128, NEM * 64], f16)
            for a0 in range(0, NEM, ECH):
                nc.sync.dma_start(w2d[:, a0 * 64:(a0 + ECH) * 64],
                                  dins["w2d"][:, a0 * 64:(a0 + ECH) * 64])
            w2d3 = w2d[:].rearrange("p (a m) -> p a m", a=NEM)
            ident = constp.tile([128, 128], f32)
            nc.sync.dma_start(ident[:], dins["ident"][:])

            for u in UORDER:
                v_sb = []
                for t, (off, rows) in enumerate(U[u]["tiles"]):
                    vt = unitp.tile([rows, DV], f32, tag=f"v{u}_{t}",
                                    name=f"v{u}_{t}")
                    nc.sync.dma_start(vt[:], dins[f"v{u}"][off:off + rows, :])
                    v_sb.append(vt)
                U[u]["v_sb"] = v_sb
                attT_sb = []
                for t, (off, rows) in enumerate(U[u]["tiles"]):
                    at = unitp.tile([rows, LQ], f32, tag=f"attT{u}_{t}",
                                    name=f"attT{u}_{t}")
                    attT_sb.append(at)
                U[u]["attT_sb"] = attT_sb

            def make_hid(u, b):
                Kp = U[u]["Kp"]
                kwrep3, qwb3 = U[u]["kwrep3"], U[u]["qwb3"]
                hid_t = hidp.tile([128, NEM, Kp], f16, tag="hid",
                                  name=f"hid{u}_{b}")
                for a0 in range(0, NEM, ECH):
                    sum_t = sump.tile([128, ECH, Kp], f32, tag="sum32",
                                      name=f"s32_{u}_{b}_{a0}")
                    nc.vector.tensor_add(
                        sum_t[:],
                        kwrep3[:, a0:a0 + ECH, :],
                        qwb3[:, b, :].unsqueeze(2)
                        .broadcast_to([128, NEM, Kp])[:, a0:a0 + ECH, :],
                    )
                    nc.scalar.activation(
                        hid_t[:, a0:a0 + ECH, :], sum_t[:], Tanh)
                return hid_t

            def do_block_pair(u, bp):
                K, Kp = U[u]["K"], U[u]["Kp"]
                tiles = U[u]["tiles"]
                b0 = 2 * bp
                hid_pair = [make_hid(u, b0), make_hid(u, b0 + 1)]
                ps_pair = ps_blk.tile([128, Kp], f32, tag="blk",
                                      name=f"blk{u}_{bp}",
                                      padded_shape=[128, 512])
                for i in (0, 1):
                    for emaj in range(NEM):
                        nc.tensor.matmul(
                            ps_pair[64 * i:64 * i + 64, :],
                            w2d3[:, emaj, :],
                            hid_pair[i][:, emaj, :],
                            start=(emaj == 0), stop=(emaj == NEM - 1),
                        )
                # softmax along k (valid K columns), straight out of PSUM
                for i in (0, 1):
                    b = b0 + i
                    sc = ps_pair[64 * i:64 * i + 64, 0:K]
                    # scores are bounded (|s| <= sum|w2| < 64), so raw exp
                    # cannot overflow fp32: skip the max-subtraction
                    att = attp.tile([64, K], f32, tag="att",
                                    name=f"att{u}_{b}")
                    sums = smallp.tile([64, 1], f32, tag="sums")
                    nc.scalar.activation(att[:], sc, Exp,
                                         accum_out=sums[:])
                    rsum = smallp.tile([64, 1], f32, tag="rsum")
                    nc.vector.reciprocal(rsum[:], sums[:])
                    nc.vector.tensor_scalar_mul(att[:], att[:], rsum[:])
                    nc.gpsimd.dma_start(
                        douts[f"att{u}"][64 * b:64 * b + 64, :], att[:])
                    for t, (off, rows) in enumerate(tiles):
                        trp = ps_tr.tile([rows, 64], f32, tag="tr",
                                         padded_shape=[128, 512])
                        nc.tensor.transpose(
                            trp[:],
                            att[:, off:off + rows],
                            ident[0:64, 0:64],
                        )
                        nc.vector.tensor_copy(
                            U[u]["attT_sb"][t][:, 64 * b:64 * b + 64],
                            trp[:])

            for bp in range(NBLK // 2):
                for u in UORDER:
                    do_block_pair(u, bp)

            for u in UORDER:
                tiles = U[u]["tiles"]
                KT = len(tiles)
                for qt in range(3):
                    op = ps_o.tile([128, DV], f32, tag="o",
                                   padded_shape=[128, 512])
                    for t, (off, rows) in enumerate(tiles):
                        nc.tensor.matmul(
                            op[:],
                            U[u]["attT_sb"][t][:, qt * 128:(qt + 1) * 128],
                            U[u]["v_sb"][t][:],
                            start=(t == 0), stop=(t == KT - 1),
                        )
                    osb = smallp.tile([128, DV], f32, tag="osb")
                    nc.vector.tensor_copy(osb[:], op[:])
                    nc.gpsimd.dma_start(
                        douts[f"o{u}"][qt * 128:(qt + 1) * 128, :], osb[:])

    nc.compile()
    return nc


def _get_program(K0, K1):
    key = (K0, K1)
    if key not in _CACHE:
        _CACHE[key] = _build_program(K0, K1)
    return _CACHE[key]


def _host_prep(q, k, v, w1, b1, w2, k_lens):
    """Per-core input maps. Core i handles head i for b=0 and b=1."""
    ident = np.eye(128, dtype=np.float32)
    in_maps = []
    for h in range(H):
        m = {"ident": ident}
        # w2d[e_sub*64+q_hat, emaj*64+m] = (q_hat==m) * w2[2*emaj+e_sub]
        w2v = w2[h].reshape(NEM, 2)  # [emaj, e_sub]
        z = np.zeros((2, 64, NEM, 64), np.float16)
        idx = np.arange(64)
        for es in range(2):
            z[es, idx, :, idx] = w2v[:, es].astype(np.float16)[None, :]
        m["w2d"] = np.ascontiguousarray(z.reshape(128, NEM * 64))
        for u in range(BS):
            K = int(k_lens[u])
            Kp = K + (K % 2)
            qh = q[u, :, h * DQ:(h + 1) * DQ]          # (384, 32)
            kh = k[u, :K, h * DK:(h + 1) * DK]         # (K, 32)
            qW = qh @ w1[h, :DQ, :] + b1[h]            # (384, 64)
            kW = kh @ w1[h, DQ:, :]                    # (K, 64)
            # kwrep[e_sub*64+q_hat, emaj*Kp+k] = kW[k, 2*emaj+e_sub]
            kw2 = kW.T.reshape(NEM, 2, K).transpose(1, 0, 2)  # (2, NEM, K)
            if Kp != K:
                kw2 = np.concatenate(
                    [kw2, np.zeros((2, NEM, Kp - K), kw2.dtype)], axis=2)
            kwrep = np.broadcast_to(
                kw2[:, None, :, :], (2, 64, NEM, Kp)).reshape(128, NEM * Kp)
            # qwb[e_sub*64+q_hat, b*NEM+emaj] = qW[64b+q_hat, 2emaj+e_sub]
            qwb = (qW.reshape(NBLK, 64, NEM, 2)
                   .transpose(3, 1, 0, 2).reshape(128, NBLK * NEM))
            m[f"kwrep{u}"] = np.ascontiguousarray(kwrep, np.float16)
            m[f"qwb{u}"] = np.ascontiguousarray(qwb, np.float32)
            m[f"v{u}"] = np.ascontiguousarray(
                v[u, :K, h * DV:(h + 1) * DV], np.float32)
        in_maps.append(m)
    return in_maps


def kernel(q, k, v, q_sequence_lengths, k_sequence_lengths, w1, b1, w2):
    from concourse.bass_utils import run_bass_kernel_spmd

    q = np.asarray(q, np.float32)
    k = np.asarray(k, np.float32)
    v = np.asarray(v, np.float32)
    w1 = np.asarray(w1, np.float32)
    b1 = np.asarray(b1, np.float32)
    w2 = np.asarray(w2, np.float32)
    q_lens = np.asarray(q_sequence_lengths).astype(np.int64)
    k_lens = np.asarray(k_sequence_lengths).astype(np.int64)

    K0, K1 = int(k_lens[0]), int(k_lens[1])
    nc = _get_program(K0, K1)
    in_maps = _host_prep(q, k, v, w1, b1, w2, k_lens)
    res = run_bass_kernel_spmd(nc, in_maps, list(range(H))).results

    att = np.zeros((BS, H, LQ, LK), np.float32)
    out = np.zeros((BS, LQ, H * DV), np.float32)
    Ks = (K0, K1)
    for h in range(H):
        for u in range(BS):
            att[u, h, :, :Ks[u]] = res[h][f"att{u}"]
            out[u, :, h * DV:(h + 1) * DV] = res[h][f"o{u}"]
    for u in range(BS):
        out[u, int(q_lens[u]):, :] = 0.0
    return out, att
